# revision 4
# baseline (speedup 1.0000x reference)
"""AddSpatialInfo: out = concat([img_feat, coord_grid], axis=1).

img_feat [64, 2048, 14, 14] f32 -> out [64, 2050, 14, 14] f32.
Data-parallel over batch: 8 cores x 8 batches. Per core the kernel is a
pure DMA problem: one big DRAM->DRAM copy of the feature shard into the
strided output, plus one tiny DRAM->DRAM copy of the NEFF-embedded coord
constant into the last two channels of each batch.
"""

import numpy as np

import concourse.bass as bass
import concourse.mybir as mybir
from concourse.bass_utils import run_bass_kernel_spmd

B, C, H, W = 64, 2048, 14, 14
HW = H * W
N_CORES = 8
BPC = B // N_CORES  # batches per core


def _coord_block() -> np.ndarray:
    # Match reference op-for-op in f32: x[j] = j*2/W - 1, y[i] = i*2/H - 1.
    xs = np.arange(W, dtype=np.float32) * np.float32(2.0) / np.float32(W) - np.float32(1.0)
    ys = np.arange(H, dtype=np.float32) * np.float32(2.0) / np.float32(H) - np.float32(1.0)
    x_ch = np.broadcast_to(xs[None, :], (H, W))
    y_ch = np.broadcast_to(ys[:, None], (H, W))
    coord = np.stack([x_ch, y_ch], axis=0)  # [2, H, W]
    return np.broadcast_to(coord[None], (BPC, 2, H, W)).reshape(BPC, 2, HW).copy()


def _build() -> bass.Bass:
    nc = bass.Bass()
    img = nc.declare_dram_parameter(
        "img_feat", [BPC, C, HW], mybir.dt.float32, isOutput=False
    )
    out = nc.declare_dram_parameter(
        "out", [BPC, C + 2, HW], mybir.dt.float32, isOutput=True
    )
    coord = nc.inline_tensor(_coord_block(), name="coord")

    with (
        nc.Block() as block,
        nc.semaphore("dma_sem") as dma_sem,
    ):

        @block.sync
        def _(sync: bass.BassEngine):
            # Single DMA with a 32-wide outer chunk dim: SDMA engine =
            # outer index % 16, so all 16 engines each get 16 descriptors
            # of 50 KB (uint16 descriptor-size limit is 64 KB).
            o4 = out[:, 0:C, :].rearrange("b (k x) s -> k b (x s)", k=32)
            i4 = img[:].rearrange("b (k x) s -> k b (x s)", k=32)
            sync.dma_start(out=o4, in_=i4).then_inc(dma_sem, 16)
            sync.dma_start(out=out[:, C : C + 2, :], in_=coord[:]).then_inc(
                dma_sem, 16
            )
            sync.wait_ge(dma_sem, 32)

    return nc


def _run(img_feat: np.ndarray, **spmd_kwargs):
    """Run on 8 cores; returns (full_output, BassKernelResults)."""
    img_feat = np.ascontiguousarray(np.asarray(img_feat, dtype=np.float32))
    nc = _build()
    in_maps = [
        {"img_feat": img_feat[i * BPC : (i + 1) * BPC].reshape(BPC, C, HW)}
        for i in range(N_CORES)
    ]
    res = run_bass_kernel_spmd(nc, in_maps, core_ids=list(range(N_CORES)), **spmd_kwargs)
    out = np.concatenate(
        [np.asarray(res.results[i]["out"]).reshape(BPC, C + 2, H, W) for i in range(N_CORES)],
        axis=0,
    )
    return out, res


def kernel(img_feat: np.ndarray) -> np.ndarray:
    out, _ = _run(img_feat)
    return out


# revision 6
# speedup vs baseline: 1.3037x; 1.3037x over previous
"""AddSpatialInfo: out = concat([img_feat, coord_grid], axis=1).

img_feat [64, 2048, 14, 14] f32 -> out [64, 2050, 14, 14] f32.
Data-parallel over batch: 8 cores x 8 batches. Per core the kernel is a
pure DMA problem: one big DRAM->DRAM copy of the feature shard into the
strided output, plus one tiny DRAM->DRAM copy of the NEFF-embedded coord
constant into the last two channels of each batch.
"""

import numpy as np

import concourse.bass as bass
import concourse.mybir as mybir
from concourse.bass_utils import run_bass_kernel_spmd

B, C, H, W = 64, 2048, 14, 14
HW = H * W
N_CORES = 8
BPC = B // N_CORES  # batches per core
# Per-batch copy (401408 f32) splits into descriptors of DESC_BYTES; the
# outer dim count (401408*4/DESC_BYTES) % 16 == 0 keeps all 16 engines even.
DESC_BYTES = 50176


def _coord_block() -> np.ndarray:
    # Match reference op-for-op in f32: x[j] = j*2/W - 1, y[i] = i*2/H - 1.
    xs = np.arange(W, dtype=np.float32) * np.float32(2.0) / np.float32(W) - np.float32(1.0)
    ys = np.arange(H, dtype=np.float32) * np.float32(2.0) / np.float32(H) - np.float32(1.0)
    x_ch = np.broadcast_to(xs[None, :], (H, W))
    y_ch = np.broadcast_to(ys[:, None], (H, W))
    coord = np.stack([x_ch, y_ch], axis=0)  # [2, H, W]
    return np.broadcast_to(coord[None], (BPC, 2, H, W)).reshape(BPC, 2, HW).copy()


def _build() -> bass.Bass:
    nc = bass.Bass()
    img = nc.declare_dram_parameter(
        "img_feat", [BPC, C, HW], mybir.dt.float32, isOutput=False
    )
    out = nc.declare_dram_parameter(
        "out", [BPC, C + 2, HW], mybir.dt.float32, isOutput=True
    )
    coord = nc.inline_tensor(_coord_block(), name="coord")

    with (
        nc.Block() as block,
        nc.semaphore("dma_sem") as dma_sem,
    ):

        @block.sync
        def _(sync: bass.BassEngine):
            # One DMA per batch; outer dim spreads across all 16 SDMA
            # engines (engine = outer index % 16).
            for b in range(BPC):
                sync.dma_start(
                    out=out[b, 0:C, :], in_=img[b], max_dma_last_dim=DESC_BYTES + 1
                ).then_inc(dma_sem, 16)
            sync.dma_start(out=out[:, C : C + 2, :], in_=coord[:]).then_inc(
                dma_sem, 16
            )
            sync.wait_ge(dma_sem, 16 * (BPC + 1))

    return nc


def _run(img_feat: np.ndarray, **spmd_kwargs):
    """Run on 8 cores; returns (full_output, BassKernelResults)."""
    img_feat = np.ascontiguousarray(np.asarray(img_feat, dtype=np.float32))
    nc = _build()
    in_maps = [
        {"img_feat": img_feat[i * BPC : (i + 1) * BPC].reshape(BPC, C, HW)}
        for i in range(N_CORES)
    ]
    res = run_bass_kernel_spmd(nc, in_maps, core_ids=list(range(N_CORES)), **spmd_kwargs)
    out = np.concatenate(
        [np.asarray(res.results[i]["out"]).reshape(BPC, C + 2, H, W) for i in range(N_CORES)],
        axis=0,
    )
    return out, res


def kernel(img_feat: np.ndarray) -> np.ndarray:
    out, _ = _run(img_feat)
    return out


# revision 7
# speedup vs baseline: 1.4404x; 1.1048x over previous
"""AddSpatialInfo: out = concat([img_feat, coord_grid], axis=1).

img_feat [64, 2048, 14, 14] f32 -> out [64, 2050, 14, 14] f32.
Data-parallel over batch: 8 cores x 8 batches. Per core the kernel is a
pure DMA problem: one big DRAM->DRAM copy of the feature shard into the
strided output, plus one tiny DRAM->DRAM copy of the NEFF-embedded coord
constant into the last two channels of each batch.
"""

import numpy as np

import concourse.bass as bass
import concourse.mybir as mybir
from concourse.bass_utils import run_bass_kernel_spmd

B, C, H, W = 64, 2048, 14, 14
HW = H * W
N_CORES = 8
BPC = B // N_CORES  # batches per core
# Per-batch copy (401408 f32) splits into descriptors of DESC_BYTES; the
# outer dim count (401408*4/DESC_BYTES) % 16 == 0 keeps all 16 engines even.
DESC_BYTES = 12544


def _coord_block() -> np.ndarray:
    # Match reference op-for-op in f32: x[j] = j*2/W - 1, y[i] = i*2/H - 1.
    xs = np.arange(W, dtype=np.float32) * np.float32(2.0) / np.float32(W) - np.float32(1.0)
    ys = np.arange(H, dtype=np.float32) * np.float32(2.0) / np.float32(H) - np.float32(1.0)
    x_ch = np.broadcast_to(xs[None, :], (H, W))
    y_ch = np.broadcast_to(ys[:, None], (H, W))
    coord = np.stack([x_ch, y_ch], axis=0)  # [2, H, W]
    return np.broadcast_to(coord[None], (BPC, 2, H, W)).reshape(BPC, 2, HW).copy()


def _build() -> bass.Bass:
    nc = bass.Bass()
    img = nc.declare_dram_parameter(
        "img_feat", [BPC, C, HW], mybir.dt.float32, isOutput=False
    )
    out = nc.declare_dram_parameter(
        "out", [BPC, C + 2, HW], mybir.dt.float32, isOutput=True
    )
    coord = nc.inline_tensor(_coord_block(), name="coord")

    with (
        nc.Block() as block,
        nc.semaphore("dma_sem") as dma_sem,
    ):

        @block.sync
        def _(sync: bass.BassEngine):
            # One DMA per batch; outer dim spreads across all 16 SDMA
            # engines (engine = outer index % 16).
            for b in range(BPC):
                sync.dma_start(
                    out=out[b, 0:C, :], in_=img[b], max_dma_last_dim=DESC_BYTES + 1
                ).then_inc(dma_sem, 16)
            sync.dma_start(out=out[:, C : C + 2, :], in_=coord[:]).then_inc(
                dma_sem, 16
            )
            sync.wait_ge(dma_sem, 16 * (BPC + 1))

    return nc


def _run(img_feat: np.ndarray, **spmd_kwargs):
    """Run on 8 cores; returns (full_output, BassKernelResults)."""
    img_feat = np.ascontiguousarray(np.asarray(img_feat, dtype=np.float32))
    nc = _build()
    in_maps = [
        {"img_feat": img_feat[i * BPC : (i + 1) * BPC].reshape(BPC, C, HW)}
        for i in range(N_CORES)
    ]
    res = run_bass_kernel_spmd(nc, in_maps, core_ids=list(range(N_CORES)), **spmd_kwargs)
    out = np.concatenate(
        [np.asarray(res.results[i]["out"]).reshape(BPC, C + 2, H, W) for i in range(N_CORES)],
        axis=0,
    )
    return out, res


def kernel(img_feat: np.ndarray) -> np.ndarray:
    out, _ = _run(img_feat)
    return out
